# revision 12
# baseline (speedup 1.0000x reference)
"""Trainium2 Bass kernel for GQA attention (nn_Attention_34832184770944).

Sharding: tensor-parallel across heads on 8 cores. Core m gets KV head m and
Q heads 4m..4m+3 (wq cols [256m,256m+256), wk/wv cols [64m,64m+64), wo rows
[256m,256m+256)).  Each core computes a full-shape partial output (row-parallel
wo); the host sums the 8 partials.

Device layout strategy:
  - host passes x transposed per batch (xT [b, d, t]) in bf16 so the
    contraction dim d lands on SBUF partitions with contiguous DMA.
  - QKV projection: lhsT = xT tile [128d,128t], rhs = wqkv [128d,384]
    -> token-major q,k,v [t,384] per tile.
  - RoPE applied token-major with stride-2 free-dim views.
  - q,k transposed per head via TensorE (identity matmul) -> qT_h [64,t].
  - scores^T tile: lhsT = kT[:,tk_tile] [64,128], rhs = qT_h [64,512]
    -> S^T [128tk, 512tq]; causal mask additively on the diagonal subtile;
    exp on ScalarE (scale=1/8, no max subtraction: |scores|*scale < ~10).
  - PV: lhsT = V_aug [128tk, 65] (col 64 = ones), rhs = P^T [128tk,512]
    -> accumulate O^T [65,512]; row 64 = softmax denominators for free.
  - normalize: recip(denom) replicated over 64 partitions with a rank-1
    matmul, then one DVE multiply -> oT_h [64,t].
  - out partial: lhsT = oT_h [64e,128t], rhs = wo_h [64e,512dm] accumulated
    over 4 heads -> [128t,512dm] fp32 -> DMA out.
"""

import os
import sys

sys.path.insert(0, "/opt/trn_rl_repo")

import numpy as np
import ml_dtypes

BF16 = ml_dtypes.bfloat16

B, S, D = 2, 2048, 2048
NH = 4              # q heads per core
HD = 64             # head dim
E = NH * HD         # 256
KD = D // 128       # 16 contraction tiles
TT = S // 128       # 16 token tiles per batch
NCH = S // 512      # 4 query chunks per batch
SCALE = 1.0 / 8.0
NEG = -1.0e9


def _build_bass():
    import concourse.bacc as bacc
    import concourse.mybir as mybir
    from concourse.tile import TileContext
    from concourse.masks import make_identity

    f32 = mybir.dt.float32
    bf16 = mybir.dt.bfloat16
    Exp = mybir.ActivationFunctionType.Exp

    nc = bacc.Bacc(None, target_bir_lowering=False)
    xT_d = nc.dram_tensor("xT", [B, D, S], bf16, kind="ExternalInput")
    wqkv_d = nc.dram_tensor("wqkv", [D, 384], bf16, kind="ExternalInput")
    wo_d = nc.dram_tensor("wo_s", [E, D], bf16, kind="ExternalInput")
    cq_d = nc.dram_tensor("cq", [S, 128], bf16, kind="ExternalInput")
    sq_d = nc.dram_tensor("sq", [S, 128], bf16, kind="ExternalInput")
    ck_d = nc.dram_tensor("ck", [S, 32], bf16, kind="ExternalInput")
    sk_d = nc.dram_tensor("sk", [S, 32], bf16, kind="ExternalInput")
    out_d = nc.dram_tensor("out", [B, S, D], f32, kind="ExternalOutput")

    with TileContext(nc) as tc:
        with (
            tc.tile_pool(name="const", bufs=1) as constp,
            tc.tile_pool(name="wts", bufs=1) as wtsp,
            tc.tile_pool(name="xin", bufs=32) as xinp,
            tc.tile_pool(name="act", bufs=1) as actp,
            tc.tile_pool(name="pt", bufs=20) as ptp,
            tc.tile_pool(name="tmp", bufs=6) as tmpp,
            tc.tile_pool(name="sml", bufs=3) as smlp,
            tc.tile_pool(name="ost", bufs=4) as ostp,
            tc.tile_pool(name="ps", bufs=8, space="PSUM") as psp,
        ):
            # ---- constants ----
            ident = constp.tile([128, 128], bf16, name="ident")
            make_identity(nc, ident[:, :])
            # transposed causal mask for S^T [tk, tq]: valid (0) where tk<=tq,
            # NEG where tk>tq:  expr = -p + f >= 0 ? keep : fill
            maskT = constp.tile([128, 128], f32, name="maskT")
            nc.gpsimd.memset(maskT[:, :], 0.0)
            nc.gpsimd.affine_select(
                out=maskT[:, :],
                in_=maskT[:, :],
                compare_op=mybir.AluOpType.is_ge,
                fill=NEG,
                base=0,
                pattern=[[1, 128]],
                channel_multiplier=-1,
            )
            ones = constp.tile([1, 64], f32, name="ones")
            nc.vector.memset(ones[:, :], 1.0)

            # ---- weights / freqs (persistent) ----
            wqkv_sb = wtsp.tile([128, KD * 384], bf16, name="wqkv_sb")
            nc.sync.dma_start(
                out=wqkv_sb[:, :].rearrange("p (k e) -> p k e", k=KD),
                in_=wqkv_d.rearrange("(k p) e -> p k e", p=128),
            )
            wo_sb = []
            for h in range(NH):
                woh = wtsp.tile([64, D], bf16, name=f"wo{h}", tag=f"wo{h}")
                nc.sync.dma_start(out=woh[:, :], in_=wo_d[h * 64 : h * 64 + 64, :])
                wo_sb.append(woh)
            cq_sb = wtsp.tile([128, TT * 128], bf16, name="cq_sb")
            nc.sync.dma_start(
                out=cq_sb[:, :].rearrange("p (n j) -> p n j", n=TT),
                in_=cq_d.rearrange("(n p) j -> p n j", p=128),
            )
            sq_sb = wtsp.tile([128, TT * 128], bf16, name="sq_sb")
            nc.sync.dma_start(
                out=sq_sb[:, :].rearrange("p (n j) -> p n j", n=TT),
                in_=sq_d.rearrange("(n p) j -> p n j", p=128),
            )
            ck_sb = wtsp.tile([128, TT * 32], bf16, name="ck_sb")
            nc.sync.dma_start(
                out=ck_sb[:, :].rearrange("p (n j) -> p n j", n=TT),
                in_=ck_d.rearrange("(n p) j -> p n j", p=128),
            )
            sk_sb = wtsp.tile([128, TT * 32], bf16, name="sk_sb")
            nc.sync.dma_start(
                out=sk_sb[:, :].rearrange("p (n j) -> p n j", n=TT),
                in_=sk_d.rearrange("(n p) j -> p n j", p=128),
            )

            # ---- per-batch activations (reused across batches) ----
            qk_sb = actp.tile([128, TT * 320], bf16, name="qk_sb")
            v_sb = actp.tile([128, TT * 68], bf16, name="v_sb")
            rot_sb = actp.tile([128, TT * 320], bf16, name="rot_sb")
            qTs = [actp.tile([64, S], bf16, name=f"qT{h}", tag=f"qT{h}") for h in range(NH)]
            kT = actp.tile([64, S], bf16, name="kT")
            oTs = [actp.tile([64, S], bf16, name=f"oT{h}", tag=f"oT{h}") for h in range(NH)]

            for b in range(B):
                # ones column of V_aug (col 64 of each 68-wide block)
                nc.vector.memset(
                    v_sb[:, :].rearrange("p (n j) -> p n j", n=TT)[:, :, 64:65], 1.0
                )

                # ---- projection: token-major q,k,v (xT in 512-token chunks) ----
                for c in range(NCH):
                    xcs = []
                    for kd in range(KD):
                        xc = xinp.tile([128, 512], bf16, tag="xc", name="xc")
                        nc.sync.dma_start(
                            out=xc[:, :],
                            in_=xT_d[
                                b, kd * 128 : kd * 128 + 128, c * 512 : c * 512 + 512
                            ],
                        )
                        xcs.append(xc)
                    for ts in range(4):
                        tt = c * 4 + ts
                        ps_qkv = psp.tile([128, 512], f32, tag="ps", name="ps_qkv")
                        for kd in range(KD):
                            nc.tensor.matmul(
                                ps_qkv[:, 0:384],
                                lhsT=xcs[kd][:, ts * 128 : ts * 128 + 128],
                                rhs=wqkv_sb[:, kd * 384 : kd * 384 + 384],
                                start=(kd == 0),
                                stop=(kd == KD - 1),
                            )
                        nc.vector.tensor_copy(
                            qk_sb[:, tt * 320 : tt * 320 + 320], ps_qkv[:, 0:320]
                        )
                        nc.vector.tensor_copy(
                            v_sb[:, tt * 68 : tt * 68 + 64], ps_qkv[:, 320:384]
                        )

                # ---- RoPE (token-major, stride-2 views) ----
                for tt in range(TT):
                    q = qk_sb[:, tt * 320 : tt * 320 + 256].rearrange(
                        "p (j two) -> p j two", two=2
                    )
                    rq = rot_sb[:, tt * 320 : tt * 320 + 256].rearrange(
                        "p (j two) -> p j two", two=2
                    )
                    c = cq_sb[:, tt * 128 : tt * 128 + 128]
                    s_ = sq_sb[:, tt * 128 : tt * 128 + 128]
                    t1 = tmpp.tile([128, 128], f32, tag="rt", name="t1")
                    t2 = tmpp.tile([128, 128], f32, tag="rt", name="t2")
                    t3 = tmpp.tile([128, 128], f32, tag="rt", name="t3")
                    t4 = tmpp.tile([128, 128], f32, tag="rt", name="t4")
                    nc.vector.tensor_mul(t1[:, :], q[:, :, 0], c)
                    nc.vector.tensor_mul(t2[:, :], q[:, :, 1], s_)
                    nc.vector.tensor_mul(t3[:, :], q[:, :, 0], s_)
                    nc.vector.tensor_mul(t4[:, :], q[:, :, 1], c)
                    nc.vector.tensor_sub(rq[:, :, 0], t1[:, :], t2[:, :])
                    nc.vector.tensor_add(rq[:, :, 1], t3[:, :], t4[:, :])

                    k = qk_sb[:, tt * 320 + 256 : tt * 320 + 320].rearrange(
                        "p (j two) -> p j two", two=2
                    )
                    rk = rot_sb[:, tt * 320 + 256 : tt * 320 + 320].rearrange(
                        "p (j two) -> p j two", two=2
                    )
                    ckc = ck_sb[:, tt * 32 : tt * 32 + 32]
                    sks = sk_sb[:, tt * 32 : tt * 32 + 32]
                    u1 = tmpp.tile([128, 32], f32, tag="ru", name="u1")
                    u2 = tmpp.tile([128, 32], f32, tag="ru", name="u2")
                    u3 = tmpp.tile([128, 32], f32, tag="ru", name="u3")
                    u4 = tmpp.tile([128, 32], f32, tag="ru", name="u4")
                    nc.vector.tensor_mul(u1[:, :], k[:, :, 0], ckc)
                    nc.vector.tensor_mul(u2[:, :], k[:, :, 1], sks)
                    nc.vector.tensor_mul(u3[:, :], k[:, :, 0], sks)
                    nc.vector.tensor_mul(u4[:, :], k[:, :, 1], ckc)
                    nc.vector.tensor_sub(rk[:, :, 0], u1[:, :], u2[:, :])
                    nc.vector.tensor_add(rk[:, :, 1], u3[:, :], u4[:, :])

                # ---- transpose q (per head) and k to e-major ----
                for tt in range(TT):
                    for h in range(NH):
                        ps_t = psp.tile([128, 512], bf16, tag="ps", name="ps_t")
                        nc.tensor.transpose(
                            ps_t[0:64, 0:128],
                            rot_sb[:, tt * 320 + h * 64 : tt * 320 + h * 64 + 64],
                            ident[:, :],
                        )
                        nc.vector.tensor_copy(
                            qTs[h][:, tt * 128 : tt * 128 + 128], ps_t[0:64, 0:128]
                        )
                    ps_k = psp.tile([128, 512], bf16, tag="ps", name="ps_k")
                    nc.tensor.transpose(
                        ps_k[0:64, 0:128],
                        rot_sb[:, tt * 320 + 256 : tt * 320 + 320],
                        ident[:, :],
                    )
                    nc.vector.tensor_copy(
                        kT[:, tt * 128 : tt * 128 + 128], ps_k[0:64, 0:128]
                    )

                # ---- attention per head / query chunk ----
                for h in range(NH):
                    for j in range(NCH):
                        qth = qTs[h][:, j * 512 : j * 512 + 512]
                        nts = 4 * j + 4
                        pts = []
                        for i in range(nts):
                            ps_s = psp.tile([128, 512], f32, tag="ps", name="ps_s")
                            nc.tensor.matmul(
                                ps_s[:, :],
                                lhsT=kT[:, i * 128 : i * 128 + 128],
                                rhs=qth,
                                start=True,
                                stop=True,
                            )
                            pt = ptp.tile([128, 512], bf16, tag="pt", name="pt")
                            pts.append(pt)
                            cd = i - 4 * j
                            if cd >= 1:
                                nc.vector.memset(pt[:, 0 : cd * 128], 0.0)
                            if cd >= 0:
                                nc.vector.tensor_add(
                                    ps_s[:, cd * 128 : cd * 128 + 128],
                                    ps_s[:, cd * 128 : cd * 128 + 128],
                                    maskT[:, :],
                                )
                            vs = max(cd, 0) * 128
                            nc.scalar.activation(
                                pt[:, vs:512], ps_s[:, vs:512], Exp, scale=SCALE
                            )
                        ps_pv = psp.tile([128, 512], f32, tag="ps", name="ps_pv")
                        for i in range(nts):
                            nc.tensor.matmul(
                                ps_pv[0:65, :],
                                lhsT=v_sb[:, i * 68 : i * 68 + 65],
                                rhs=pts[i][:, :],
                                start=(i == 0),
                                stop=(i == nts - 1),
                            )
                        dn = smlp.tile([1, 512], f32, tag="dn", name="dn")
                        nc.vector.tensor_copy(dn[:, :], ps_pv[64:65, :])
                        rd = smlp.tile([1, 512], f32, tag="rd", name="rd")
                        nc.vector.reciprocal(rd[:, :], dn[:, :])
                        ps_rep = psp.tile([128, 512], f32, tag="ps", name="ps_rep")
                        nc.tensor.matmul(
                            ps_rep[0:64, :],
                            lhsT=ones[0:1, 0:64],
                            rhs=rd[:, :],
                            start=True,
                            stop=True,
                        )
                        ob = tmpp.tile([64, 512], bf16, tag="ob", name="ob")
                        nc.vector.tensor_copy(ob[:, :], ps_pv[0:64, :])
                        nc.vector.tensor_mul(
                            oTs[h][:, j * 512 : j * 512 + 512],
                            ob[:, :],
                            ps_rep[0:64, :],
                        )

                # ---- output projection (partial; host sums across cores) ----
                for tt in range(TT):
                    psos = [
                        psp.tile([128, 512], f32, tag="ps", name="ps_o")
                        for _ in range(4)
                    ]
                    for h in range(NH):
                        for dmc in range(4):
                            nc.tensor.matmul(
                                psos[dmc][:, :],
                                lhsT=oTs[h][:, tt * 128 : tt * 128 + 128],
                                rhs=wo_sb[h][:, dmc * 512 : dmc * 512 + 512],
                                start=(h == 0),
                                stop=(h == NH - 1),
                            )
                    for dmc in range(4):
                        ot = ostp.tile([128, 512], f32, tag="ot", name="ot")
                        nc.vector.tensor_copy(ot[:, :], psos[dmc][:, :])
                        nc.sync.dma_start(
                            out=out_d[
                                b, tt * 128 : tt * 128 + 128, dmc * 512 : dmc * 512 + 512
                            ],
                            in_=ot[:, :],
                        )
    nc.compile()
    return nc


def _prep_in_maps(inputs):
    x = np.asarray(inputs["x"], dtype=np.float32)
    fc = np.asarray(inputs["freqs_cos"], dtype=np.float32)
    fs = np.asarray(inputs["freqs_sin"], dtype=np.float32)
    wq = np.asarray(inputs["wq"], dtype=np.float32)
    wk = np.asarray(inputs["wk"], dtype=np.float32)
    wv = np.asarray(inputs["wv"], dtype=np.float32)
    wo = np.asarray(inputs["wo"], dtype=np.float32)

    xT = np.ascontiguousarray(np.transpose(x, (0, 2, 1))).astype(BF16)
    cqh = np.ascontiguousarray(np.tile(fc, (1, 4))).astype(BF16)  # [S,128]
    sqh = np.ascontiguousarray(np.tile(fs, (1, 4))).astype(BF16)
    ckh = np.ascontiguousarray(fc).astype(BF16)  # [S,32]
    skh = np.ascontiguousarray(fs).astype(BF16)

    in_maps = []
    for m in range(8):
        wqs = wq[:, m * 256 : m * 256 + 256]
        wks = wk[:, m * 64 : m * 64 + 64]
        wvs = wv[:, m * 64 : m * 64 + 64]
        wqkv = np.ascontiguousarray(
            np.concatenate([wqs, wks, wvs], axis=1)
        ).astype(BF16)
        wos = np.ascontiguousarray(wo[m * 256 : m * 256 + 256, :]).astype(BF16)
        in_maps.append(
            dict(xT=xT, wqkv=wqkv, wo_s=wos, cq=cqh, sq=sqh, ck=ckh, sk=skh)
        )
    return in_maps


def kernel(**inputs):
    from concourse import bass_utils

    in_maps = _prep_in_maps(inputs)
    nc = _build_bass()
    trace = bool(int(os.environ.get("KERNEL_TRACE", "0")))
    res = bass_utils.run_bass_kernel_spmd(
        nc, in_maps, core_ids=list(range(8)), trace=trace
    )
    if trace and res.exec_time_ns is not None:
        print(f"HW exec time: {res.exec_time_ns} ns")
    out = np.zeros((B, S, D), dtype=np.float32)
    for r in res.results:
        out += r["out"]
    return out


def time_device(reps=6, **inputs):
    """Wall-clock the sharded PJRT executable with device-resident inputs.

    Returns the min wall time in ns across `reps` runs (first run after
    compile is discarded).  This is an upper bound on HW exec time (adds
    axon RPC dispatch overhead) but has no host<->device transfer in the
    timed region.
    """
    import jax
    from concourse import bass2jax
    import concourse.mybir as mybir
    import time as _time

    in_maps = _prep_in_maps(inputs)
    nc = _build_bass()
    bass2jax.install_neuronx_cc_hook()

    partition_name = (
        nc.partition_id_tensor.name if nc.partition_id_tensor else None
    )
    in_names, out_names, out_avals, zero_outs = [], [], [], []
    for alloc in nc.m.functions[0].allocations:
        if not isinstance(alloc, mybir.MemoryLocationSet):
            continue
        name = alloc.memorylocations[0].name
        if alloc.kind == "ExternalInput":
            if name != partition_name:
                in_names.append(name)
        elif alloc.kind == "ExternalOutput":
            out_names.append(name)
            shape = tuple(alloc.tensor_shape)
            dt = mybir.dt.np(alloc.dtype)
            out_avals.append(jax.core.ShapedArray(shape, dt))
            zero_outs.append(np.zeros(shape, dt))
    n_params = len(in_names)
    in_all = in_names + out_names
    if partition_name is not None:
        in_all = in_all + [partition_name]

    def _body(*args):
        operands = list(args)
        if partition_name is not None:
            operands.append(bass2jax.partition_id_tensor())
        outs = bass2jax._bass_exec_p.bind(
            *operands,
            out_avals=tuple(out_avals),
            in_names=tuple(in_all),
            out_names=tuple(out_names),
            lowering_input_output_aliases=(),
            sim_require_finite=True,
            sim_require_nnan=True,
            nc=nc,
        )
        return tuple(outs)

    devices = jax.devices()[:8]
    mesh = bass2jax.Mesh(np.asarray(devices), ("core",))
    spec = bass2jax.PartitionSpec("core")
    nin = n_params + len(out_names)
    f = jax.jit(
        bass2jax.shard_map(
            _body,
            mesh=mesh,
            in_specs=(spec,) * nin,
            out_specs=(spec,) * len(out_names),
            check_rep=False,
        )
    )
    concat_in = [
        np.concatenate([np.asarray(m[n]) for m in in_maps], axis=0)
        for n in in_names
    ]
    concat_zeros = [
        np.zeros((8 * z.shape[0], *z.shape[1:]), z.dtype) for z in zero_outs
    ]
    sharding = jax.sharding.NamedSharding(mesh, spec)
    dev_args = [jax.device_put(a, sharding) for a in concat_in + concat_zeros]
    r = f(*dev_args)
    jax.block_until_ready(r)
    best = None
    for _ in range(reps):
        t0 = _time.perf_counter()
        r = f(*dev_args)
        jax.block_until_ready(r)
        dt = _time.perf_counter() - t0
        best = dt if best is None else min(best, dt)
    return int(best * 1e9)
